# revision 1
# baseline (speedup 1.0000x reference)
"""Trainium2 Bass kernel for nn_FMAPModelWarping (retrieval_knn).

The host does the cheap per-pixel prep (affine grids, bilinear taps, the
3x3x3->64 and 1x1 convs, 4-tap backward warp — ~1 GFLOP total, <4% of the
model) and ships fp8 feature maps. The 8 NeuronCores do the FLOP-heavy
part (~26.5 GFLOP): two 3600x3600x128 correlations per sample and the
bidirectional-softmax reduction, tiled flash-attention-style.

Sharding: core k = 2*b + s handles sample b (of 4) and row-half s of the
3600x3600 correlation matrices; partial column stats combine on the host.

Math restructure (exact):
  g[n] = 1/U_h[n],  res_sum[m] = O[m] / U_v[m],
  O[m] = sum_n g[n] * eh[n,m] * ev[n,m]
with U_h = rowsum(eh), U_v = colsum(ev), eh = exp(Mh), ev = exp(Mv).

Device structure: features live in a dual-plane [64, 2, n] fp8 layout so
the correlation matmuls run in DoubleRow perf mode (256-deep contraction,
0.5 cycles/row). Phase R computes exp(Mh) row-blocks (kept in SBUF) with
the row sums coming free from the activation engine's accumulator; phase F
(m-outer) recomputes exp(Mv), forms t = eh*ev, and accumulates O and U_v
via PSUM matvecs. R-chunks and F-chunks share a two-deep PSUM rotation and
are interleaved so the ACT exp stream, DVE/Pool elementwise work and PE
matmuls all overlap; part of the exp(Mv) field uses a Schraudolph bit-trick
exponential on DVE (its small relative noise cancels between O and U_v,
which consume the same ev values).
"""

import numpy as np

B, C_IN, H, W = 4, 3, 60, 60
HID, FEAT = 64, 128
N = H * W               # 3600
NCORES = 8
HALF = N // 2           # 1800 rows per core
NBLK = 128              # correlation row-block (partition dim)
NNB = 15                # row blocks per core (15*128 = 1920, rows padded)
NPAD = NNB * NBLK       # 1920
MT = 450                # m-tile width
N_MT = N // MT          # 8 m tiles
BANKW = 512             # fp32 elems per PSUM bank

# Schraudolph constants (bf16 target): i16 = rne(x*SA + SB); bits as bf16.
SA = 128.0 / float(np.log(2.0))
SB = 127.0 * 128.0 - 5.5 - 1.86   # -1.86 centers the measured +1% bias

# m-tile groups sharing the PSUM accumulator banks (4 rows at partition
# offsets 0/32/64/96 per bank; O and U_v each get one bank).
SWEEPJS = [(0, 1, 2, 3), (4, 5, 6, 7)]


# ----------------------------------------------------------------------------
# Host-side prep: exact reference semantics for grids / bilinear taps / rolls
# ----------------------------------------------------------------------------

def _affine_coords(theta2x3):
    """Pixel-space sample coords (x, y) for torch affine_grid+grid_sample
    (align_corners=False), shape [H, W] each."""
    xs = (2.0 * np.arange(W, dtype=np.float64) + 1.0) / W - 1.0
    ys = (2.0 * np.arange(H, dtype=np.float64) + 1.0) / H - 1.0
    gx, gy = np.meshgrid(xs, ys)           # gx[i,j]=xs[j], gy[i,j]=ys[i]
    t = theta2x3.astype(np.float64)
    cx = t[0, 0] * gx + t[0, 1] * gy + t[0, 2]
    cy = t[1, 0] * gx + t[1, 1] * gy + t[1, 2]
    px = (cx + 1.0) * W * 0.5 - 0.5
    py = (cy + 1.0) * H * 0.5 - 0.5
    return px, py


def _bilinear_sample_host(img, px, py):
    """img [C,H,W] float32, sample at (px,py) [H,W]; zeros padding.
    Mirrors reference grid_sample exactly."""
    x0 = np.floor(px); y0 = np.floor(py)
    wx1 = (px - x0); wx0 = 1.0 - wx1
    wy1 = (py - y0); wy0 = 1.0 - wy1
    out = np.zeros((img.shape[0],) + px.shape, np.float64)
    flat = img.reshape(img.shape[0], -1).astype(np.float64)
    for ix, iy, wt in ((x0, y0, wx0 * wy0), (x0 + 1, y0, wx1 * wy0),
                       (x0, y0 + 1, wx0 * wy1), (x0 + 1, y0 + 1, wx1 * wy1)):
        valid = (ix >= 0) & (ix < W) & (iy >= 0) & (iy < H)
        ii = np.clip(ix, 0, W - 1).astype(np.int64)
        jj = np.clip(iy, 0, H - 1).astype(np.int64)
        v = flat[:, (jj * W + ii).ravel()].reshape(out.shape)
        out += v * (wt * valid)[None]
    return out.astype(np.float32)


def _back_taps(theta2x3, u, v):
    """Tap indices/weights for grid_sample(y, grid(Bm)) composed with the
    inverse roll. Returns idx [4,3600] int (in-range), wt [4,3600] f32."""
    px, py = _affine_coords(theta2x3)
    ii = np.arange(H)[:, None]; jj = np.arange(W)[None, :]
    qi = (ii - u) % H; qj = (jj - v) % W
    xs = px[qi, qj].ravel(); ys = py[qi, qj].ravel()
    x0 = np.floor(xs); y0 = np.floor(ys)
    fx = xs - x0; fy = ys - y0
    idxs, wts = [], []
    for ix, iy, wt in ((x0, y0, (1 - fx) * (1 - fy)), (x0 + 1, y0, fx * (1 - fy)),
                       (x0, y0 + 1, (1 - fx) * fy), (x0 + 1, y0 + 1, fx * fy)):
        valid = (ix >= 0) & (ix < W) & (iy >= 0) & (iy < H)
        cii = np.clip(ix, 0, W - 1).astype(np.int64)
        cjj = np.clip(iy, 0, H - 1).astype(np.int64)
        idxs.append(cjj * W + cii)
        wts.append((wt * valid).astype(np.float32))
    return np.stack(idxs), np.stack(wts)


def _host_prep(inputs):
    """Build the 8 per-core device input dicts (fp8 dual-plane features)."""
    import ml_dtypes
    x_a = np.asarray(inputs["input_a"], np.float32)
    x_b = np.asarray(inputs["input_b"], np.float32)
    w1 = np.asarray(inputs["w1"], np.float32)
    b1 = np.asarray(inputs["b1"], np.float32)
    w2 = np.asarray(inputs["w2"], np.float32)
    b2 = np.asarray(inputs["b2"], np.float32)
    noise = np.asarray(inputs["noise"], np.float32)
    u_roll = np.asarray(inputs["u_roll"])
    v_roll = np.asarray(inputs["v_roll"])
    swap = np.asarray(inputs["swap"])

    w1mat = w1.reshape(HID, C_IN * 9)                  # [64, 27]
    w2mat = w2.reshape(FEAT, HID)                      # [128, 64]

    eye = np.eye(3, dtype=np.float64)
    mask = np.array([[1., 1., 1.], [1., 1., 1.], [0., 0., 0.]])

    # F[wrp][b]: warped feature map [FEAT, N] float32 (exact reference math;
    # the 1x1 conv2 commutes with the backward spatial gather)
    F = np.zeros((4, B, FEAT, N), np.float32)
    for wrp in range(4):
        sw = int(swap[wrp]) == 1
        for b in range(B):
            fwd = eye + 0.05 * noise[wrp, b].astype(np.float64) * mask
            bwd = np.linalg.inv(fwd)
            A_ = bwd if sw else fwd
            Bm = fwd if sw else bwd
            u = int(u_roll[wrp, b]); v = int(v_roll[wrp, b])
            img = x_a[b] if wrp in (0, 2) else x_b[b]
            x_r = np.roll(np.roll(img, -u, axis=1), -v, axis=2)
            px, py = _affine_coords(np.asarray(A_)[:2])
            xw = _bilinear_sample_host(x_r, px, py)       # [3,60,60]
            # im2col, zero-pad SAME, k = c*9 + ky*3 + kx
            pad = np.zeros((C_IN, H + 2, W + 2), np.float32)
            pad[:, 1:-1, 1:-1] = xw
            X1 = np.zeros((C_IN * 9, N), np.float32)
            k = 0
            for c in range(C_IN):
                for ky in range(3):
                    for kx in range(3):
                        X1[k] = pad[c, ky:ky + H, kx:kx + W].ravel()
                        k += 1
            y1 = np.maximum(w1mat @ X1 + b1[:, None], 0.0)   # [64, N]
            y2 = w2mat @ y1 + b2[:, None]                    # [128, N]
            idx, wt = _back_taps(np.asarray(Bm)[:2], u, v)
            Fw = np.zeros((FEAT, N), np.float32)
            for tap in range(4):
                Fw += y2[:, idx[tap]] * wt[tap][None, :]
            F[wrp, b] = Fw

    F8 = F.astype(ml_dtypes.float8_e4m3fn)

    def dual_plane(feat, cols):
        """[FEAT, n] -> [64, 2, cols] (zero-padded)."""
        out = np.zeros((64, 2, cols), ml_dtypes.float8_e4m3fn)
        n = feat.shape[1]
        out[:, 0, :n] = feat[0:64]
        out[:, 1, :n] = feat[64:128]
        return out

    # U_v matvec stationaries: ones, except block 14 masks the 120 pad rows
    onesmask = np.ones((128, NNB), np.float32)
    onesmask[8:, NNB - 1] = 0.0

    in_maps = []
    for core in range(NCORES):
        b = core // 2
        s = core % 2
        n0 = s * HALF
        in_maps.append({
            "f8ah_in": dual_plane(F8[0, b][:, n0:n0 + HALF], NPAD),
            "f8av_in": dual_plane(F8[2, b][:, n0:n0 + HALF], NPAD),
            "f8bh_in": dual_plane(F8[1, b], N),
            "f8bv_in": dual_plane(F8[3, b], N),
            "onesmask_in": onesmask.astype(ml_dtypes.bfloat16),
        })
    return in_maps


# ----------------------------------------------------------------------------
# Device kernel builder
# ----------------------------------------------------------------------------

_CACHED = {}


def _build(core_half):
    """Build the Bacc module (one NEFF shared by all 8 cores; each core's
    sample/row-half is fully encoded in its host-built feature tiles)."""
    import concourse.bacc as bacc_mod
    import concourse.mybir as mybir
    from concourse.tile import TileContext
    from contextlib import ExitStack
    import itertools

    dt = mybir.dt
    Alu = mybir.AluOpType
    Act = mybir.ActivationFunctionType
    DR = mybir.MatmulPerfMode.DoubleRow

    nc = bacc_mod.Bacc("TRN2", target_bir_lowering=False)

    f8ah_in = nc.dram_tensor("f8ah_in", [64, 2, NPAD], dt.float8e4, kind="ExternalInput")
    f8av_in = nc.dram_tensor("f8av_in", [64, 2, NPAD], dt.float8e4, kind="ExternalInput")
    f8bh_in = nc.dram_tensor("f8bh_in", [64, 2, N], dt.float8e4, kind="ExternalInput")
    f8bv_in = nc.dram_tensor("f8bv_in", [64, 2, N], dt.float8e4, kind="ExternalInput")
    onesmask_in = nc.dram_tensor("onesmask_in", [FEAT, NNB], dt.bfloat16, kind="ExternalInput")

    o_out = nc.dram_tensor("o_out", [2, 4, MT], dt.float32, kind="ExternalOutput")
    uv_out = nc.dram_tensor("uv_out", [2, 4, MT], dt.float32, kind="ExternalOutput")

    with ExitStack() as ctx:
        tc = ctx.enter_context(TileContext(nc))

        const = ctx.enter_context(tc.tile_pool(name="const", bufs=1))
        onesmask_t = const.tile([FEAT, NNB], dt.bfloat16)
        nc.sync.dma_start(onesmask_t[:], onesmask_in[:])

        fpool = ctx.enter_context(tc.tile_pool(name="feat", bufs=1))
        f8ah = fpool.tile([64, 2, NPAD], dt.float8e4, name="f8ah")
        f8av = fpool.tile([64, 2, NPAD], dt.float8e4, name="f8av")
        f8bh = fpool.tile([64, 2, N], dt.float8e4, name="f8bh")
        f8bv = fpool.tile([64, 2, N], dt.float8e4, name="f8bv")
        nc.sync.dma_start(f8ah[:], f8ah_in[:])
        nc.sync.dma_start(f8bh[:], f8bh_in[:])
        nc.sync.dma_start(f8av[:], f8av_in[:])
        nc.sync.dma_start(f8bv[:], f8bv_in[:])

        stat = ctx.enter_context(tc.tile_pool(name="stat", bufs=1))
        eh_t = [stat.tile([NBLK, N], dt.bfloat16, name=f"eh{_nb}")
                for _nb in range(NNB)]
        g_t = [stat.tile([NBLK, 1], dt.bfloat16, name=f"g{_nb}")
               for _nb in range(NNB)]

        rs = ctx.enter_context(tc.tile_pool(name="rsmall", bufs=10))

        # Single PSUM pool, bank budget 8: tag "u" (2 x 3 banks, a unified
        # two-deep rotation shared by R chunks and F chunks — consecutive
        # chunks land in alternating buffers so each chunk's matmuls overlap
        # the previous chunk's exp), plus two accumulator banks (4 rows each
        # at partition offsets 0/32/64/96).
        pz = ctx.enter_context(tc.tile_pool(name="pz", bufs=1, space="PSUM"))
        fwkpool = ctx.enter_context(tc.tile_pool(name="fwkpool", bufs=1))

        oacc = pz.tile([NBLK, BANKW], dt.float32, tag="accA", name="oacc")
        uvacc = pz.tile([NBLK, BANKW], dt.float32, tag="accB", name="uvacc")

        # R chunking: m ranges as (start, n_tiles) with 450-wide tiles
        RCH = [(0, 3), (1350, 3), (2700, 2)]

        def emit_r_chunk(nb, ci):
            nsl = slice(nb * NBLK, (nb + 1) * NBLK)
            m0, nt = RCH[ci]
            rt = pz.tile([NBLK, 3, BANKW], dt.float32, tag="u", bufs=2,
                         name=f"r_{nb}_{ci}")
            for k in range(nt):
                nc.tensor.matmul(rt[:, k, 0:MT], f8ah[:, :, nsl],
                                 f8bh[:, :, m0 + k * MT: m0 + (k + 1) * MT],
                                 start=True, stop=True, perf_mode=DR)
            ehv = eh_t[nb][:, m0: m0 + nt * MT] \
                .rearrange("p (c w) -> p c w", w=MT)
            uh = rs.tile([NBLK, 1], dt.float32, tag=f"uh{ci}",
                         name=f"uh_{nb}_{ci}")
            nc.scalar.activation(ehv, rt[:, 0:nt, 0:MT], Act.Exp,
                                 accum_out=uh[:])
            return uh

        def emit_r_gfin(nb, uhp):
            ua = rs.tile([NBLK, 1], dt.float32, tag="ua", name=f"ua_{nb}")
            nc.vector.tensor_tensor(ua[:], uhp[0][:], uhp[1][:], Alu.add)
            nc.vector.tensor_tensor(ua[:], ua[:], uhp[2][:], Alu.add)
            gr = rs.tile([NBLK, 1], dt.float32, tag="gr", name=f"gr_{nb}")
            nc.vector.reciprocal(gr[:], ua[:])
            nc.vector.tensor_copy(g_t[nb][:], gr[:])
            if nb == NNB - 1:   # zero g on the 120 pad rows
                nc.vector.tensor_tensor(g_t[nb][:], g_t[nb][:],
                                        onesmask_t[:, nb:nb + 1], Alu.mult)

        # ---------------- phase F chunk ---------------------------------
        JPOS = {}
        JSWEEP = {}
        for _s, _js in enumerate(SWEEPJS):
            for _p, _jv in enumerate(_js):
                JPOS[_jv] = _p
                JSWEEP[_jv] = _s

        def emit_f_front(j, nbc, ev_eng, t_eng):
            jsl = slice(j * MT, (j + 1) * MT)
            nbs = [3 * nbc + k for k in range(3)]
            ft = pz.tile([NBLK, 3, BANKW], dt.float32, tag="u", bufs=2,
                         name=f"f_{j}_{nbc}")
            for kk, nb in enumerate(nbs):
                nsl = slice(nb * NBLK, (nb + 1) * NBLK)
                nc.tensor.matmul(ft[:, kk, 0:MT], f8av[:, :, nsl],
                                 f8bv[:, :, jsl], start=True, stop=True,
                                 perf_mode=DR)
            ev_i = fwkpool.tile([NBLK, 3, MT], dt.int16, tag="ev", bufs=6,
                                name=f"ev_{j}_{nbc}")
            evb = ev_i[:].bitcast(dt.bfloat16)
            if ev_eng == "act":
                nc.scalar.activation(evb, ft[:, :, 0:MT], Act.Exp)
            else:
                nc.vector.tensor_scalar(ev_i[:], ft[:, :, 0:MT], SA, SB,
                                        Alu.mult, Alu.add)
            t_t = fwkpool.tile([NBLK, 3, MT], dt.bfloat16, tag="t", bufs=6,
                               name=f"t_{j}_{nbc}")
            teng = nc.vector if t_eng == "dve" else nc.gpsimd
            for kk, nb in enumerate(nbs):
                teng.tensor_tensor(t_t[:, kk, :], eh_t[nb][:, jsl],
                                   ev_i[:, kk, :].bitcast(dt.bfloat16),
                                   Alu.mult)
            return (j, nbc, t_t, ev_i)

        def emit_f_mvs(front):
            j, nbc, t_t, ev_i = front
            jj = JPOS[j]
            nbs = [3 * nbc + k for k in range(3)]
            orow = oacc[32 * jj:32 * jj + 1, 0:MT]
            uvrow = uvacc[32 * jj:32 * jj + 1, 0:MT]
            for kk, nb in enumerate(nbs):
                nc.tensor.matmul(orow, g_t[nb][:], t_t[:, kk, :],
                                 start=(nb == 0), stop=(nb == NNB - 1),
                                 skip_group_check=True,
                                 tile_position=(0, 32 * jj))
                nc.tensor.matmul(uvrow, onesmask_t[:, nb:nb + 1],
                                 ev_i[:, kk, :].bitcast(dt.bfloat16),
                                 start=(nb == 0), stop=(nb == NNB - 1),
                                 skip_group_check=True,
                                 tile_position=(0, 32 * jj))

        def emit_sweep_drain(s):
            osb = fwkpool.tile([128, MT], dt.float32, tag="osb", bufs=1,
                               name=f"osb_{s}")
            uvsb = fwkpool.tile([128, MT], dt.float32, tag="uvsb", bufs=1,
                                name=f"uvsb_{s}")
            nc.vector.tensor_copy(osb[:], oacc[:, 0:MT])
            nc.vector.tensor_copy(uvsb[:], uvacc[:, 0:MT])
            ov = osb[:].rearrange("(q t) m -> q t m", t=32)[:, 0, :]
            uvv = uvsb[:].rearrange("(q t) m -> q t m", t=32)[:, 0, :]
            nc.sync.dma_start(o_out[s], ov)
            nc.sync.dma_start(uv_out[s], uvv)

        # ---------------- schedule --------------------------------------
        # F-chunk queue: sweep-major, then nbc-major within sweep
        fqueue = [(s, j, c) for s, js in enumerate(SWEEPJS)
                  for c in range(5) for j in js]
        f_next = 0
        pending = []       # emitted fronts awaiting their matvecs
        g_done = -1
        drained = -1       # last sweep whose accumulators were drained
        ev_alt = itertools.cycle(["act", "act", "dve"])
        ev_alt_r = itertools.cycle(["dve"])
        t_alt = itertools.cycle(["dve", "dve", "dve", "pool"])

        def f_ready():
            if f_next >= len(fqueue):
                return False
            s, j, c = fqueue[f_next]
            if g_done < 3 * c + 2:
                return False
            return True

        def maybe_drain():
            nonlocal drained
            s = drained + 1
            if s >= len(SWEEPJS):
                return
            n_done = sum(1 for i in range(f_next)
                         if fqueue[i][0] == s) - sum(1 for fr in pending
                                                    if JSWEEP[fr[0]] == s)
            if n_done == 5 * len(SWEEPJS[s]):
                emit_sweep_drain(s)
                drained = s

        def f_slot(during_r):
            nonlocal f_next
            lag = 4
            if (len(pending) >= lag + (1 if f_ready() else 0)) or \
                    (pending and not f_ready()):
                emit_f_mvs(pending.pop(0))
                maybe_drain()
            if f_ready():
                s, j, c = fqueue[f_next]
                eng = next(ev_alt_r) if during_r else next(ev_alt)
                pending.append(emit_f_front(j, c, eng, next(t_alt)))
                f_next += 1

        for nb in range(NNB):
            uhp = []
            for ci in range(3):
                uhp.append(emit_r_chunk(nb, ci))
                f_slot(during_r=True)
            emit_r_gfin(nb, uhp)
            g_done = nb
        while f_next < len(fqueue) or pending:
            f_slot(during_r=False)
        while drained < len(SWEEPJS) - 1:
            maybe_drain()

    nc.compile()
    return nc


def _get_nc(s):
    if s not in _CACHED:
        _CACHED[s] = _build(s)
    return _CACHED[s]


# ----------------------------------------------------------------------------
# Entry point
# ----------------------------------------------------------------------------

def kernel(**inputs):
    from concourse.bass_utils import run_bass_kernel_spmd

    in_maps = _host_prep(inputs)

    # One program for all 8 cores: the sample/row-half each core handles is
    # fully encoded in its host-built feature tiles.
    nc = _get_nc(0)
    last_err = None
    for attempt in range(3):
        try:
            r = run_bass_kernel_spmd(nc, in_maps, core_ids=list(range(NCORES)))
            break
        except Exception as e:  # transient NRT_EXEC_UNIT_UNRECOVERABLE wedges
            last_err = e
            import time
            time.sleep(10 * (attempt + 1))
    else:
        raise last_err
    results = r.results

    # host combine (exact)
    def _gather_m(arr):
        out = np.zeros(N, np.float64)
        for s, js in enumerate(SWEEPJS):
            for p, j in enumerate(js):
                out[j * MT:(j + 1) * MT] = arr[s, p].astype(np.float64)
        return out

    logs = np.zeros((B, N), np.float64)
    for b in range(B):
        r0, r1 = results[2 * b], results[2 * b + 1]
        O = _gather_m(r0["o_out"]) + _gather_m(r1["o_out"])
        uv = _gather_m(r0["uv_out"]) + _gather_m(r1["uv_out"])
        res_sum = O / uv
        logs[b] = np.log(res_sum + 1e-4)
    return np.float32(logs.mean())



# revision 23
# speedup vs baseline: 1.1410x; 1.1410x over previous
"""Trainium2 Bass kernel for nn_FMAPModelWarping (retrieval_knn).

The host does the cheap per-pixel prep (affine grids, bilinear taps, the
3x3x3->64 and 1x1 convs, 4-tap backward warp — ~1 GFLOP total, <4% of the
model) and ships fp8 feature maps. The 8 NeuronCores do the FLOP-heavy
part (~26.5 GFLOP): two 3600x3600x128 correlations per sample and the
bidirectional-softmax reduction, tiled flash-attention-style.

Sharding: core k = 2*b + s handles sample b (of 4) and row-half s of the
3600x3600 correlation matrices; partial column stats combine on the host.

Math restructure (exact):
  g[n] = 1/U_h[n],  res_sum[m] = O[m] / U_v[m],
  O[m] = sum_n g[n] * eh[n,m] * ev[n,m]
with U_h = rowsum(eh), U_v = colsum(ev), eh = exp(Mh), ev = exp(Mv).

Device structure: features live in a dual-plane [64, 2, n] fp8 layout so
the correlation matmuls run in DoubleRow perf mode.  Phase R (ACT-bound)
computes exp(Mh) row-blocks into SBUF on its own 4-bank double-buffered
PSUM rotation — no other engine ever blocks it.  Phase F computes exp(Mv)
per (row-pair, m-tile) chunk — Schraudolph bit-trick exponentials on Pool
(+DVE/ACT after R drains) — forms t = eh*ev on DVE, reduces O over rows
via PSUM-accumulated PE matvecs (all 8 m-tiles live at once, 2 banks),
and ships the raw bf16 exp(Mv) tiles to HBM over the otherwise-idle DMA
engines: the column sums U_v are finished on the host, which removes the
whole U_v reduction from the device's critical engines.  The Schraudolph
noise cancels between O and U_v since both consume identical ev values.
"""

import numpy as np

B, C_IN, H, W = 4, 3, 60, 60
HID, FEAT = 64, 128
N = H * W               # 3600
NCORES = 8
HALF = N // 2           # 1800 rows per core
NBLK = 128              # correlation row-block (partition dim)
NNB = 15                # row blocks per core (15*128 = 1920, rows padded)
NPAD = NNB * NBLK       # 1920
MT = 450                # m-tile width
N_MT = N // MT          # 8 m tiles
BANKW = 512             # fp32 elems per PSUM bank
NFC = 120               # F chunks: 15 row-blocks x 8 m-tiles

# Schraudolph constants (bf16 target): i16 = rne(x*SA + SB); bits as bf16.
SA = 128.0 / float(np.log(2.0))
SB = 127.0 * 128.0 - 5.5 - 1.86   # -1.86 centers the measured +1% bias

# m-tile groups: bank A holds O rows for j=0..3, bank B for j=4..7 (4 rows
# per bank at partition offsets 0/32/64/96); the same grouping indexes the
# [2, 4, MT] output.
SWEEPJS = [(0, 1, 2, 3), (4, 5, 6, 7)]

# F-chunk queue: row-block-major, m-tile-minor (all 8 m-tiles' O rows
# accumulate concurrently in two PSUM banks).
FQUEUE = [(nb, j) for nb in range(NNB) for j in range(8)]

# scheduling knobs (tuned via cost-model sweeps)
_FSLOT_PAT = (2,)       # F slots per R chunk, cycled
_LAG = 12       # fronts in flight before their matvecs are emitted
_EVBUFS = 16    # ev/t tile rotation depth
# Pool cannot read PSUM (BIR verifier), so exp(Mv) runs on DVE with the
# tail of the chunk queue on ACT (free once phase R drains); Pool instead
# carries most of the SBUF-only t=eh*ev multiplies.
_EV_ACT_TAIL = 104     # chunks with fidx >= this use ACT for exp(Mv)
_T_ENG = ["pool", "pool", "dve"]         # t-mult engine cycle


# ----------------------------------------------------------------------------
# Host-side prep: exact reference semantics for grids / bilinear taps / rolls
# ----------------------------------------------------------------------------

def _affine_coords(theta2x3):
    """Pixel-space sample coords (x, y) for torch affine_grid+grid_sample
    (align_corners=False), shape [H, W] each."""
    xs = (2.0 * np.arange(W, dtype=np.float64) + 1.0) / W - 1.0
    ys = (2.0 * np.arange(H, dtype=np.float64) + 1.0) / H - 1.0
    gx, gy = np.meshgrid(xs, ys)           # gx[i,j]=xs[j], gy[i,j]=ys[i]
    t = theta2x3.astype(np.float64)
    cx = t[0, 0] * gx + t[0, 1] * gy + t[0, 2]
    cy = t[1, 0] * gx + t[1, 1] * gy + t[1, 2]
    px = (cx + 1.0) * W * 0.5 - 0.5
    py = (cy + 1.0) * H * 0.5 - 0.5
    return px, py


def _bilinear_sample_host(img, px, py):
    """img [C,H,W] float32, sample at (px,py) [H,W]; zeros padding.
    Mirrors reference grid_sample exactly."""
    x0 = np.floor(px); y0 = np.floor(py)
    wx1 = (px - x0); wx0 = 1.0 - wx1
    wy1 = (py - y0); wy0 = 1.0 - wy1
    out = np.zeros((img.shape[0],) + px.shape, np.float64)
    flat = img.reshape(img.shape[0], -1).astype(np.float64)
    for ix, iy, wt in ((x0, y0, wx0 * wy0), (x0 + 1, y0, wx1 * wy0),
                       (x0, y0 + 1, wx0 * wy1), (x0 + 1, y0 + 1, wx1 * wy1)):
        valid = (ix >= 0) & (ix < W) & (iy >= 0) & (iy < H)
        ii = np.clip(ix, 0, W - 1).astype(np.int64)
        jj = np.clip(iy, 0, H - 1).astype(np.int64)
        v = flat[:, (jj * W + ii).ravel()].reshape(out.shape)
        out += v * (wt * valid)[None]
    return out.astype(np.float32)


def _back_taps(theta2x3, u, v):
    """Tap indices/weights for grid_sample(y, grid(Bm)) composed with the
    inverse roll. Returns idx [4,3600] int (in-range), wt [4,3600] f32."""
    px, py = _affine_coords(theta2x3)
    ii = np.arange(H)[:, None]; jj = np.arange(W)[None, :]
    qi = (ii - u) % H; qj = (jj - v) % W
    xs = px[qi, qj].ravel(); ys = py[qi, qj].ravel()
    x0 = np.floor(xs); y0 = np.floor(ys)
    fx = xs - x0; fy = ys - y0
    idxs, wts = [], []
    for ix, iy, wt in ((x0, y0, (1 - fx) * (1 - fy)), (x0 + 1, y0, fx * (1 - fy)),
                       (x0, y0 + 1, (1 - fx) * fy), (x0 + 1, y0 + 1, fx * fy)):
        valid = (ix >= 0) & (ix < W) & (iy >= 0) & (iy < H)
        cii = np.clip(ix, 0, W - 1).astype(np.int64)
        cjj = np.clip(iy, 0, H - 1).astype(np.int64)
        idxs.append(cjj * W + cii)
        wts.append((wt * valid).astype(np.float32))
    return np.stack(idxs), np.stack(wts)


def _host_prep(inputs):
    """Build the 8 per-core device input dicts (fp8 dual-plane features)."""
    import ml_dtypes
    x_a = np.asarray(inputs["input_a"], np.float32)
    x_b = np.asarray(inputs["input_b"], np.float32)
    w1 = np.asarray(inputs["w1"], np.float32)
    b1 = np.asarray(inputs["b1"], np.float32)
    w2 = np.asarray(inputs["w2"], np.float32)
    b2 = np.asarray(inputs["b2"], np.float32)
    noise = np.asarray(inputs["noise"], np.float32)
    u_roll = np.asarray(inputs["u_roll"])
    v_roll = np.asarray(inputs["v_roll"])
    swap = np.asarray(inputs["swap"])

    w1mat = w1.reshape(HID, C_IN * 9)                  # [64, 27]
    w2mat = w2.reshape(FEAT, HID)                      # [128, 64]

    eye = np.eye(3, dtype=np.float64)
    mask = np.array([[1., 1., 1.], [1., 1., 1.], [0., 0., 0.]])

    # F[wrp][b]: warped feature map [FEAT, N] float32 (exact reference math;
    # the 1x1 conv2 commutes with the backward spatial gather)
    F = np.zeros((4, B, FEAT, N), np.float32)
    for wrp in range(4):
        sw = int(swap[wrp]) == 1
        for b in range(B):
            fwd = eye + 0.05 * noise[wrp, b].astype(np.float64) * mask
            bwd = np.linalg.inv(fwd)
            A_ = bwd if sw else fwd
            Bm = fwd if sw else bwd
            u = int(u_roll[wrp, b]); v = int(v_roll[wrp, b])
            img = x_a[b] if wrp in (0, 2) else x_b[b]
            x_r = np.roll(np.roll(img, -u, axis=1), -v, axis=2)
            px, py = _affine_coords(np.asarray(A_)[:2])
            xw = _bilinear_sample_host(x_r, px, py)       # [3,60,60]
            # im2col, zero-pad SAME, k = c*9 + ky*3 + kx
            pad = np.zeros((C_IN, H + 2, W + 2), np.float32)
            pad[:, 1:-1, 1:-1] = xw
            X1 = np.zeros((C_IN * 9, N), np.float32)
            k = 0
            for c in range(C_IN):
                for ky in range(3):
                    for kx in range(3):
                        X1[k] = pad[c, ky:ky + H, kx:kx + W].ravel()
                        k += 1
            y1 = np.maximum(w1mat @ X1 + b1[:, None], 0.0)   # [64, N]
            y2 = w2mat @ y1 + b2[:, None]                    # [128, N]
            idx, wt = _back_taps(np.asarray(Bm)[:2], u, v)
            Fw = np.zeros((FEAT, N), np.float32)
            for tap in range(4):
                Fw += y2[:, idx[tap]] * wt[tap][None, :]
            F[wrp, b] = Fw

    F8 = F.astype(ml_dtypes.float8_e4m3fn)

    def dual_plane(feat, cols):
        """[FEAT, n] -> [64, 2, cols] (zero-padded)."""
        out = np.zeros((64, 2, cols), ml_dtypes.float8_e4m3fn)
        n = feat.shape[1]
        out[:, 0, :n] = feat[0:64]
        out[:, 1, :n] = feat[64:128]
        return out

    # g-mask stationary: ones, except block 14 masks the 120 pad rows
    onesmask = np.ones((128, NNB), np.float32)
    onesmask[8:, NNB - 1] = 0.0

    in_maps = []
    for core in range(NCORES):
        b = core // 2
        s = core % 2
        n0 = s * HALF
        in_maps.append({
            "f8ah_in": dual_plane(F8[0, b][:, n0:n0 + HALF], NPAD),
            "f8av_in": dual_plane(F8[2, b][:, n0:n0 + HALF], NPAD),
            "f8bh_in": dual_plane(F8[1, b], N),
            "f8bv_in": dual_plane(F8[3, b], N),
            "onesmask_in": onesmask.astype(ml_dtypes.bfloat16),
        })
    return in_maps


# ----------------------------------------------------------------------------
# Device kernel builder
# ----------------------------------------------------------------------------

_CACHED = {}


def _build(core_half):
    """Build the Bacc module (one NEFF shared by all 8 cores; each core's
    sample/row-half is fully encoded in its host-built feature tiles)."""
    import concourse.bacc as bacc_mod
    import concourse.mybir as mybir
    from concourse.tile import TileContext
    from contextlib import ExitStack
    import itertools

    dt = mybir.dt
    Alu = mybir.AluOpType
    Act = mybir.ActivationFunctionType
    DR = mybir.MatmulPerfMode.DoubleRow

    nc = bacc_mod.Bacc("TRN2", target_bir_lowering=False)

    f8ah_in = nc.dram_tensor("f8ah_in", [64, 2, NPAD], dt.float8e4, kind="ExternalInput")
    f8av_in = nc.dram_tensor("f8av_in", [64, 2, NPAD], dt.float8e4, kind="ExternalInput")
    f8bh_in = nc.dram_tensor("f8bh_in", [64, 2, N], dt.float8e4, kind="ExternalInput")
    f8bv_in = nc.dram_tensor("f8bv_in", [64, 2, N], dt.float8e4, kind="ExternalInput")
    onesmask_in = nc.dram_tensor("onesmask_in", [FEAT, NNB], dt.bfloat16, kind="ExternalInput")

    o_out = nc.dram_tensor("o_out", [2, 4, MT], dt.float32, kind="ExternalOutput")
    ev_out = nc.dram_tensor("ev_out", [NFC // 8, NBLK, 8 * MT], dt.bfloat16, kind="ExternalOutput")

    with ExitStack() as ctx:
        tc = ctx.enter_context(TileContext(nc))

        const = ctx.enter_context(tc.tile_pool(name="const", bufs=1))
        onesmask_t = const.tile([FEAT, NNB], dt.bfloat16)
        nc.sync.dma_start(onesmask_t[:], onesmask_in[:])

        fpool = ctx.enter_context(tc.tile_pool(name="feat", bufs=1))
        f8ah = fpool.tile([64, 2, NPAD], dt.float8e4, name="f8ah")
        f8av = fpool.tile([64, 2, NPAD], dt.float8e4, name="f8av")
        f8bh = fpool.tile([64, 2, N], dt.float8e4, name="f8bh")
        f8bv = fpool.tile([64, 2, N], dt.float8e4, name="f8bv")
        nc.sync.dma_start(f8ah[:], f8ah_in[:])
        nc.sync.dma_start(f8bh[:], f8bh_in[:])
        nc.sync.dma_start(f8av[:], f8av_in[:])
        nc.sync.dma_start(f8bv[:], f8bv_in[:])

        stat = ctx.enter_context(tc.tile_pool(name="stat", bufs=1))
        eh_big = stat.tile([NBLK, NNB, N], dt.bfloat16, name="ehbig")
        g_t = [stat.tile([NBLK, 1], dt.bfloat16, name=f"g{_nb}")
               for _nb in range(NNB)]

        rs = ctx.enter_context(tc.tile_pool(name="rsmall", bufs=10))

        # PSUM (8 banks): uR = R-phase rotation, 2-bank chunks x 2 bufs
        # (4 banks) — R's ACT stream never waits on any other engine; uF =
        # F-phase rotation, 2-bank chunks, single buf during R (serial,
        # Pool-paced) — after R retires, F chunks also cycle through uR's
        # freed banks for a 3-deep rotation; oaccA/oaccB hold the 8 live
        # O-accumulator rows (4 rows per bank at partition offsets
        # 0/32/64/96) so the whole F phase runs c-major with no
        # inter-m-tile serialization.
        pz = ctx.enter_context(tc.tile_pool(name="pz", bufs=1, space="PSUM"))
        fwkpool = ctx.enter_context(tc.tile_pool(name="fwkpool", bufs=1))

        oaccA = pz.tile([NBLK, BANKW], dt.float32, tag="accA", name="oaccA")
        oaccB = pz.tile([NBLK, BANKW], dt.float32, tag="accB", name="oaccB")

        # R chunking: 4 chunks per nb, 2 m-tiles (900 cols) each
        def emit_r_chunk(nb, ci, uhp):
            nsl = slice(nb * NBLK, (nb + 1) * NBLK)
            m0 = ci * 2 * MT
            rt = pz.tile([NBLK, 2, BANKW], dt.float32, tag="uR", bufs=2,
                         name=f"r_{nb}_{ci}")
            for k in range(2):
                nc.tensor.matmul(rt[:, k, 0:MT], f8ah[:, :, nsl],
                                 f8bh[:, :, m0 + k * MT: m0 + (k + 1) * MT],
                                 start=True, stop=True, perf_mode=DR)
            ehv = eh_big[:, nb, m0: m0 + 2 * MT] \
                .rearrange("p (c w) -> p c w", w=MT)
            uh = rs.tile([NBLK, 1], dt.float32, tag=f"uh{ci}",
                         name=f"uh_{nb}_{ci}")
            nc.scalar.activation(ehv, rt[:, 0:2, 0:MT], Act.Exp,
                                 accum_out=uh[:])
            uhp.append(uh)

        def emit_r_gfin(nb, uhp):
            ua = rs.tile([NBLK, 1], dt.float32, tag="ua", name=f"ua_{nb}")
            nc.vector.tensor_tensor(ua[:], uhp[0][:], uhp[1][:], Alu.add)
            nc.vector.tensor_tensor(ua[:], ua[:], uhp[2][:], Alu.add)
            nc.vector.tensor_tensor(ua[:], ua[:], uhp[3][:], Alu.add)

            gr = rs.tile([NBLK, 1], dt.float32, tag="gr", name=f"gr_{nb}")
            nc.vector.reciprocal(gr[:], ua[:])
            nc.vector.tensor_copy(g_t[nb][:], gr[:])
            if nb == NNB - 1:   # zero g on the 120 pad rows
                nc.vector.tensor_tensor(g_t[nb][:], g_t[nb][:],
                                        onesmask_t[:, nb:nb + 1], Alu.mult)

        # ---------------- phase F chunk ---------------------------------

        ev_grp = [None]

        def emit_f_front(fidx, nb, j, ev_eng, tag):
            jsl = slice(j * MT, (j + 1) * MT)
            nsl = slice(nb * NBLK, (nb + 1) * NBLK)
            if tag == "uR":
                ft2 = pz.tile([NBLK, 2, BANKW], dt.float32, tag="uR",
                              bufs=2, name=f"fr_{nb}_{j}")
                ft = ft2[:, 0, :]
            else:
                ft = pz.tile([NBLK, BANKW], dt.float32, tag=tag,
                             bufs=2, name=f"f_{nb}_{j}")[:]
            nc.tensor.matmul(ft[0:NBLK, 0:MT], f8av[:, :, nsl],
                             f8bv[:, :, jsl], start=True, stop=True,
                             perf_mode=DR)
            # ev tiles stage in groups of 8 and ship to HBM as ONE DMA per
            # group (HWDGE issue overhead is per-instruction); U_v column
            # sums finish on the host.
            slot = fidx % 8
            if slot == 0:
                ev_grp[0] = fwkpool.tile([NBLK, 8, MT], dt.bfloat16, tag="ev",
                                         bufs=3, name=f"evg_{fidx // 8}")
            evb = ev_grp[0][:, slot, :]
            ev_i = evb.bitcast(dt.int16)
            if ev_eng == "act":
                nc.scalar.activation(evb, ft[0:NBLK, 0:MT], Act.Exp)
            elif ev_eng == "pool":
                nc.gpsimd.tensor_scalar(ev_i, ft[0:NBLK, 0:MT], SA, SB,
                                        Alu.mult, Alu.add)
            else:
                nc.vector.tensor_scalar(ev_i, ft[0:NBLK, 0:MT], SA, SB,
                                        Alu.mult, Alu.add)
            if slot == 7:
                nc.sync.dma_start(ev_out[fidx // 8],
                                  ev_grp[0][:].rearrange("p a m -> p (a m)"))
            # t = eh * ev (SBUF-only, so Pool can carry most of these)
            t_t = fwkpool.tile([NBLK, MT], dt.bfloat16, tag="t",
                               bufs=_EVBUFS, name=f"t_{nb}_{j}")
            teng = nc.gpsimd if next(t_eng_cycle) == "pool" else nc.vector
            teng.tensor_tensor(t_t[:], eh_big[:, nb, jsl], evb,
                               Alu.mult)
            return (nb, j, t_t)

        def emit_f_mvs(front):
            nb, j, t_t = front
            jj = j % 4
            obank = oaccA if j < 4 else oaccB
            orow = obank[32 * jj:32 * jj + 1, 0:MT]
            nc.tensor.matmul(orow, g_t[nb][:], t_t[:],
                             start=(nb == 0), stop=(nb == NNB - 1),
                             skip_group_check=True,
                             tile_position=(0, 32 * jj))
            if nb == NNB - 1:
                emit_group_drain(j // 4, jj)

        osb = fwkpool.tile([128, 2, MT], dt.float32, tag="osb", bufs=1,
                           name="osb")
        grp_done = [0, 0]

        def emit_group_drain(grp, jj):
            # row jj of bank grp finished; after all 4, copy + DMA the rows.
            grp_done[grp] += 1
            if grp_done[grp] < 4:
                return
            bank = oaccA if grp == 0 else oaccB
            nc.vector.tensor_copy(osb[:, grp, :], bank[:, 0:MT])
            ov = osb[:, grp, :].rearrange("(q t) m -> q t m", t=32)[:, 0, :]
            nc.sync.dma_start(o_out[grp], ov)

        # ---------------- schedule --------------------------------------
        f_next = 0
        pending = []       # emitted fronts awaiting their matvecs
        g_done = -1
        r_emitted = 0
        t_eng_cycle = itertools.cycle(_T_ENG)
        tag_post = itertools.cycle(["uF"])

        def f_ready():
            if f_next >= len(FQUEUE):
                return False
            nb, j = FQUEUE[f_next]
            return g_done >= nb

        def f_slot(during_r, tag_r="uF"):
            nonlocal f_next
            if (len(pending) >= _LAG + (1 if f_ready() else 0)) or \
                    (pending and not f_ready()):
                emit_f_mvs(pending.pop(0))
            if f_ready():
                nb, j = FQUEUE[f_next]
                eng = "act" if f_next >= _EV_ACT_TAIL else "dve"
                tag = tag_r if during_r else next(tag_post)
                pending.append(emit_f_front(f_next, nb, j, eng, tag))
                f_next += 1

        for nb in range(NNB):
            uhp = []
            for ci in range(4):
                emit_r_chunk(nb, ci, uhp)
                for _ in range(_FSLOT_PAT[r_emitted % len(_FSLOT_PAT)]):
                    f_slot(during_r=True)
                if r_emitted % 2 == 1:  # third F chain through uR's banks
                    f_slot(during_r=True, tag_r="uR")
                r_emitted += 1
            emit_r_gfin(nb, uhp)
            g_done = nb
        while f_next < len(FQUEUE) or pending:
            f_slot(during_r=False)

    nc.compile()
    return nc


def _get_nc(s):
    if s not in _CACHED:
        _CACHED[s] = _build(s)
    return _CACHED[s]


# ----------------------------------------------------------------------------
# Entry point
# ----------------------------------------------------------------------------

def kernel(**inputs):
    from concourse.bass_utils import run_bass_kernel_spmd

    in_maps = _host_prep(inputs)

    # One program for all 8 cores: the sample/row-half each core handles is
    # fully encoded in its host-built feature tiles.
    nc = _get_nc(0)
    last_err = None
    for attempt in range(3):
        try:
            r = run_bass_kernel_spmd(nc, in_maps, core_ids=list(range(NCORES)))
            break
        except Exception as e:  # transient NRT_EXEC_UNIT_UNRECOVERABLE wedges
            last_err = e
            import time
            time.sleep(10 * (attempt + 1))
    else:
        raise last_err
    results = r.results

    # host combine (exact)
    def _gather_o(arr):
        out = np.zeros(N, np.float64)
        for s, js in enumerate(SWEEPJS):
            for p, j in enumerate(js):
                out[j * MT:(j + 1) * MT] = arr[s, p].astype(np.float64)
        return out

    def _uv_from_ev(ev):
        """ev [NFC//8, 128, 8, MT] bf16 -> U_v partial [N] (core's rows)."""
        uv = np.zeros(N, np.float64)
        e = ev.astype(np.float64)
        for fidx, (nb, j) in enumerate(FQUEUE):
            jsl = slice(j * MT, (j + 1) * MT)
            tile = e[fidx // 8, :, (fidx % 8) * MT:(fidx % 8 + 1) * MT]
            if nb == NNB - 1:   # only partitions 0..7 are real rows
                uv[jsl] += tile[0:8].sum(axis=0)
            else:
                uv[jsl] += tile.sum(axis=0)
        return uv

    logs = np.zeros((B, N), np.float64)
    for b in range(B):
        r0, r1 = results[2 * b], results[2 * b + 1]
        O = _gather_o(r0["o_out"]) + _gather_o(r1["o_out"])
        uv = _uv_from_ev(r0["ev_out"]) + _uv_from_ev(r1["ev_out"])
        res_sum = O / uv
        logs[b] = np.log(res_sum + 1e-4)
    return np.float32(logs.mean())


# revision 26
# speedup vs baseline: 1.1677x; 1.0233x over previous
"""Trainium2 Bass kernel for nn_FMAPModelWarping (retrieval_knn).

The host does the cheap per-pixel prep (affine grids, bilinear taps, the
3x3x3->64 and 1x1 convs, 4-tap backward warp — ~1 GFLOP total, <4% of the
model) and ships fp8 feature maps. The 8 NeuronCores do the FLOP-heavy
part (~26.5 GFLOP): two 3600x3600x128 correlations per sample and the
bidirectional-softmax reduction, tiled flash-attention-style.

Sharding: core k = 2*b + s handles sample b (of 4) and row-half s of the
3600x3600 correlation matrices; partial column stats combine on the host.

Math restructure (exact):
  g[n] = 1/U_h[n],  res_sum[m] = O[m] / U_v[m],
  O[m] = sum_n g[n] * eh[n,m] * ev[n,m]
with U_h = rowsum(eh), U_v = colsum(ev), eh = exp(Mh), ev = exp(Mv).

Device structure: features live in a dual-plane [64, 2, n] fp8 layout so
the correlation matmuls run in DoubleRow perf mode.  Phase R (ACT-bound)
computes exp(Mh) row-blocks into SBUF on its own 4-bank double-buffered
PSUM rotation — no other engine ever blocks it.  Phase F computes exp(Mv)
per (row-pair, m-tile) chunk — Schraudolph bit-trick exponentials on Pool
(+DVE/ACT after R drains) — forms t = eh*ev on DVE, reduces O over rows
via PSUM-accumulated PE matvecs (all 8 m-tiles live at once, 2 banks),
and ships the raw bf16 exp(Mv) tiles to HBM over the otherwise-idle DMA
engines: the column sums U_v are finished on the host, which removes the
whole U_v reduction from the device's critical engines.  The Schraudolph
noise cancels between O and U_v since both consume identical ev values.
"""

import numpy as np

B, C_IN, H, W = 4, 3, 60, 60
HID, FEAT = 64, 128
N = H * W               # 3600
NCORES = 8
HALF = N // 2           # 1800 rows per core
NBLK = 128              # correlation row-block (partition dim)
NNB = 15                # row blocks per core (15*128 = 1920, rows padded)
NPAD = NNB * NBLK       # 1920
MT = 450                # m-tile width
N_MT = N // MT          # 8 m tiles
BANKW = 512             # fp32 elems per PSUM bank
NFC = 120               # F chunks: 15 row-blocks x 8 m-tiles

# Schraudolph constants (bf16 target): i16 = rne(x*SA + SB); bits as bf16.
SA = 128.0 / float(np.log(2.0))
SB = 127.0 * 128.0 - 5.5 - 1.86   # -1.86 centers the measured +1% bias

# m-tile groups: bank A holds O rows for j=0..3, bank B for j=4..7 (4 rows
# per bank at partition offsets 0/32/64/96); the same grouping indexes the
# [2, 4, MT] output.
SWEEPJS = [(0, 1, 2, 3), (4, 5, 6, 7)]

# F-chunk queue: row-block-major, m-tile-minor (all 8 m-tiles' O rows
# accumulate concurrently in two PSUM banks).
FQUEUE = [(nb, j) for nb in range(NNB) for j in range(8)]

# scheduling knobs (tuned via cost-model sweeps)
_FSLOT_PAT = (2,)       # F slots per R chunk, cycled
_LAG = 12       # fronts in flight before their matvecs are emitted
_EVBUFS = 16    # ev/t tile rotation depth
# Pool cannot read PSUM (BIR verifier), so exp(Mv) runs on DVE with the
# tail of the chunk queue on ACT (free once phase R drains); Pool instead
# carries most of the SBUF-only t=eh*ev multiplies.
_EV_ACT_TAIL = 80      # chunks with fidx >= this rotate through _TAIL_ENG
_TAIL_ENG = ["act", "dve", "dve"]
_MID_ACT_STRIDE = 8    # if >0, every Nth pre-tail chunk exps on ACT
_URCHAIN = True        # third F chain through uR's banks
_T_ENG = ["pool", "pool", "dve"]         # t-mult engine cycle


# ----------------------------------------------------------------------------
# Host-side prep: exact reference semantics for grids / bilinear taps / rolls
# ----------------------------------------------------------------------------

def _affine_coords(theta2x3):
    """Pixel-space sample coords (x, y) for torch affine_grid+grid_sample
    (align_corners=False), shape [H, W] each."""
    xs = (2.0 * np.arange(W, dtype=np.float64) + 1.0) / W - 1.0
    ys = (2.0 * np.arange(H, dtype=np.float64) + 1.0) / H - 1.0
    gx, gy = np.meshgrid(xs, ys)           # gx[i,j]=xs[j], gy[i,j]=ys[i]
    t = theta2x3.astype(np.float64)
    cx = t[0, 0] * gx + t[0, 1] * gy + t[0, 2]
    cy = t[1, 0] * gx + t[1, 1] * gy + t[1, 2]
    px = (cx + 1.0) * W * 0.5 - 0.5
    py = (cy + 1.0) * H * 0.5 - 0.5
    return px, py


def _bilinear_sample_host(img, px, py):
    """img [C,H,W] float32, sample at (px,py) [H,W]; zeros padding.
    Mirrors reference grid_sample exactly."""
    x0 = np.floor(px); y0 = np.floor(py)
    wx1 = (px - x0); wx0 = 1.0 - wx1
    wy1 = (py - y0); wy0 = 1.0 - wy1
    out = np.zeros((img.shape[0],) + px.shape, np.float64)
    flat = img.reshape(img.shape[0], -1).astype(np.float64)
    for ix, iy, wt in ((x0, y0, wx0 * wy0), (x0 + 1, y0, wx1 * wy0),
                       (x0, y0 + 1, wx0 * wy1), (x0 + 1, y0 + 1, wx1 * wy1)):
        valid = (ix >= 0) & (ix < W) & (iy >= 0) & (iy < H)
        ii = np.clip(ix, 0, W - 1).astype(np.int64)
        jj = np.clip(iy, 0, H - 1).astype(np.int64)
        v = flat[:, (jj * W + ii).ravel()].reshape(out.shape)
        out += v * (wt * valid)[None]
    return out.astype(np.float32)


def _back_taps(theta2x3, u, v):
    """Tap indices/weights for grid_sample(y, grid(Bm)) composed with the
    inverse roll. Returns idx [4,3600] int (in-range), wt [4,3600] f32."""
    px, py = _affine_coords(theta2x3)
    ii = np.arange(H)[:, None]; jj = np.arange(W)[None, :]
    qi = (ii - u) % H; qj = (jj - v) % W
    xs = px[qi, qj].ravel(); ys = py[qi, qj].ravel()
    x0 = np.floor(xs); y0 = np.floor(ys)
    fx = xs - x0; fy = ys - y0
    idxs, wts = [], []
    for ix, iy, wt in ((x0, y0, (1 - fx) * (1 - fy)), (x0 + 1, y0, fx * (1 - fy)),
                       (x0, y0 + 1, (1 - fx) * fy), (x0 + 1, y0 + 1, fx * fy)):
        valid = (ix >= 0) & (ix < W) & (iy >= 0) & (iy < H)
        cii = np.clip(ix, 0, W - 1).astype(np.int64)
        cjj = np.clip(iy, 0, H - 1).astype(np.int64)
        idxs.append(cjj * W + cii)
        wts.append((wt * valid).astype(np.float32))
    return np.stack(idxs), np.stack(wts)


def _host_prep(inputs):
    """Build the 8 per-core device input dicts (fp8 dual-plane features)."""
    import ml_dtypes
    x_a = np.asarray(inputs["input_a"], np.float32)
    x_b = np.asarray(inputs["input_b"], np.float32)
    w1 = np.asarray(inputs["w1"], np.float32)
    b1 = np.asarray(inputs["b1"], np.float32)
    w2 = np.asarray(inputs["w2"], np.float32)
    b2 = np.asarray(inputs["b2"], np.float32)
    noise = np.asarray(inputs["noise"], np.float32)
    u_roll = np.asarray(inputs["u_roll"])
    v_roll = np.asarray(inputs["v_roll"])
    swap = np.asarray(inputs["swap"])

    w1mat = w1.reshape(HID, C_IN * 9)                  # [64, 27]
    w2mat = w2.reshape(FEAT, HID)                      # [128, 64]

    eye = np.eye(3, dtype=np.float64)
    mask = np.array([[1., 1., 1.], [1., 1., 1.], [0., 0., 0.]])

    # F[wrp][b]: warped feature map [FEAT, N] float32 (exact reference math;
    # the 1x1 conv2 commutes with the backward spatial gather)
    F = np.zeros((4, B, FEAT, N), np.float32)
    for wrp in range(4):
        sw = int(swap[wrp]) == 1
        for b in range(B):
            fwd = eye + 0.05 * noise[wrp, b].astype(np.float64) * mask
            bwd = np.linalg.inv(fwd)
            A_ = bwd if sw else fwd
            Bm = fwd if sw else bwd
            u = int(u_roll[wrp, b]); v = int(v_roll[wrp, b])
            img = x_a[b] if wrp in (0, 2) else x_b[b]
            x_r = np.roll(np.roll(img, -u, axis=1), -v, axis=2)
            px, py = _affine_coords(np.asarray(A_)[:2])
            xw = _bilinear_sample_host(x_r, px, py)       # [3,60,60]
            # im2col, zero-pad SAME, k = c*9 + ky*3 + kx
            pad = np.zeros((C_IN, H + 2, W + 2), np.float32)
            pad[:, 1:-1, 1:-1] = xw
            X1 = np.zeros((C_IN * 9, N), np.float32)
            k = 0
            for c in range(C_IN):
                for ky in range(3):
                    for kx in range(3):
                        X1[k] = pad[c, ky:ky + H, kx:kx + W].ravel()
                        k += 1
            y1 = np.maximum(w1mat @ X1 + b1[:, None], 0.0)   # [64, N]
            y2 = w2mat @ y1 + b2[:, None]                    # [128, N]
            idx, wt = _back_taps(np.asarray(Bm)[:2], u, v)
            Fw = np.zeros((FEAT, N), np.float32)
            for tap in range(4):
                Fw += y2[:, idx[tap]] * wt[tap][None, :]
            F[wrp, b] = Fw

    F8 = F.astype(ml_dtypes.float8_e4m3fn)

    def dual_plane(feat, cols):
        """[FEAT, n] -> [64, 2, cols] (zero-padded)."""
        out = np.zeros((64, 2, cols), ml_dtypes.float8_e4m3fn)
        n = feat.shape[1]
        out[:, 0, :n] = feat[0:64]
        out[:, 1, :n] = feat[64:128]
        return out

    # g-mask stationary: ones, except block 14 masks the 120 pad rows
    onesmask = np.ones((128, NNB), np.float32)
    onesmask[8:, NNB - 1] = 0.0

    in_maps = []
    for core in range(NCORES):
        b = core // 2
        s = core % 2
        n0 = s * HALF
        in_maps.append({
            "f8ah_in": dual_plane(F8[0, b][:, n0:n0 + HALF], NPAD),
            "f8av_in": dual_plane(F8[2, b][:, n0:n0 + HALF], NPAD),
            "f8bh_in": dual_plane(F8[1, b], N),
            "f8bv_in": dual_plane(F8[3, b], N),
            "onesmask_in": onesmask.astype(ml_dtypes.bfloat16),
        })
    return in_maps


# ----------------------------------------------------------------------------
# Device kernel builder
# ----------------------------------------------------------------------------

_CACHED = {}


def _build(core_half):
    """Build the Bacc module (one NEFF shared by all 8 cores; each core's
    sample/row-half is fully encoded in its host-built feature tiles)."""
    import concourse.bacc as bacc_mod
    import concourse.mybir as mybir
    from concourse.tile import TileContext
    from contextlib import ExitStack
    import itertools

    dt = mybir.dt
    Alu = mybir.AluOpType
    Act = mybir.ActivationFunctionType
    DR = mybir.MatmulPerfMode.DoubleRow

    nc = bacc_mod.Bacc("TRN2", target_bir_lowering=False)

    f8ah_in = nc.dram_tensor("f8ah_in", [64, 2, NPAD], dt.float8e4, kind="ExternalInput")
    f8av_in = nc.dram_tensor("f8av_in", [64, 2, NPAD], dt.float8e4, kind="ExternalInput")
    f8bh_in = nc.dram_tensor("f8bh_in", [64, 2, N], dt.float8e4, kind="ExternalInput")
    f8bv_in = nc.dram_tensor("f8bv_in", [64, 2, N], dt.float8e4, kind="ExternalInput")
    onesmask_in = nc.dram_tensor("onesmask_in", [FEAT, NNB], dt.bfloat16, kind="ExternalInput")

    o_out = nc.dram_tensor("o_out", [2, 4, MT], dt.float32, kind="ExternalOutput")
    ev_out = nc.dram_tensor("ev_out", [NFC // 8, NBLK, 8 * MT], dt.bfloat16, kind="ExternalOutput")

    with ExitStack() as ctx:
        tc = ctx.enter_context(TileContext(nc))

        const = ctx.enter_context(tc.tile_pool(name="const", bufs=1))
        onesmask_t = const.tile([FEAT, NNB], dt.bfloat16)
        nc.sync.dma_start(onesmask_t[:], onesmask_in[:])

        fpool = ctx.enter_context(tc.tile_pool(name="feat", bufs=1))
        f8ah = fpool.tile([64, 2, NPAD], dt.float8e4, name="f8ah")
        f8av = fpool.tile([64, 2, NPAD], dt.float8e4, name="f8av")
        f8bh = fpool.tile([64, 2, N], dt.float8e4, name="f8bh")
        f8bv = fpool.tile([64, 2, N], dt.float8e4, name="f8bv")
        nc.sync.dma_start(f8ah[:], f8ah_in[:])
        nc.sync.dma_start(f8bh[:], f8bh_in[:])
        nc.sync.dma_start(f8av[:], f8av_in[:])
        nc.sync.dma_start(f8bv[:], f8bv_in[:])

        stat = ctx.enter_context(tc.tile_pool(name="stat", bufs=1))
        eh_big = stat.tile([NBLK, NNB, N], dt.bfloat16, name="ehbig")
        g_t = [stat.tile([NBLK, 1], dt.bfloat16, name=f"g{_nb}")
               for _nb in range(NNB)]

        rs = ctx.enter_context(tc.tile_pool(name="rsmall", bufs=10))

        # PSUM (8 banks): uR = R-phase rotation, 2-bank chunks x 2 bufs
        # (4 banks) — R's ACT stream never waits on any other engine; uF =
        # F-phase rotation, 2-bank chunks, single buf during R (serial,
        # Pool-paced) — after R retires, F chunks also cycle through uR's
        # freed banks for a 3-deep rotation; oaccA/oaccB hold the 8 live
        # O-accumulator rows (4 rows per bank at partition offsets
        # 0/32/64/96) so the whole F phase runs c-major with no
        # inter-m-tile serialization.
        pz = ctx.enter_context(tc.tile_pool(name="pz", bufs=1, space="PSUM"))
        fwkpool = ctx.enter_context(tc.tile_pool(name="fwkpool", bufs=1))

        oaccA = pz.tile([NBLK, BANKW], dt.float32, tag="accA", name="oaccA")
        oaccB = pz.tile([NBLK, BANKW], dt.float32, tag="accB", name="oaccB")

        # R chunking: 4 chunks per nb, 2 m-tiles (900 cols) each
        def emit_r_chunk(nb, ci, uhp):
            nsl = slice(nb * NBLK, (nb + 1) * NBLK)
            m0 = ci * 2 * MT
            rt = pz.tile([NBLK, 2, BANKW], dt.float32, tag="uR", bufs=2,
                         name=f"r_{nb}_{ci}")
            for k in range(2):
                nc.tensor.matmul(rt[:, k, 0:MT], f8ah[:, :, nsl],
                                 f8bh[:, :, m0 + k * MT: m0 + (k + 1) * MT],
                                 start=True, stop=True, perf_mode=DR)
            ehv = eh_big[:, nb, m0: m0 + 2 * MT] \
                .rearrange("p (c w) -> p c w", w=MT)
            uh = rs.tile([NBLK, 1], dt.float32, tag=f"uh{ci}",
                         name=f"uh_{nb}_{ci}")
            nc.scalar.activation(ehv, rt[:, 0:2, 0:MT], Act.Exp,
                                 accum_out=uh[:])
            uhp.append(uh)

        def emit_r_gfin(nb, uhp):
            ua = rs.tile([NBLK, 1], dt.float32, tag="ua", name=f"ua_{nb}")
            nc.vector.tensor_tensor(ua[:], uhp[0][:], uhp[1][:], Alu.add)
            nc.vector.tensor_tensor(ua[:], ua[:], uhp[2][:], Alu.add)
            nc.vector.tensor_tensor(ua[:], ua[:], uhp[3][:], Alu.add)

            gr = rs.tile([NBLK, 1], dt.float32, tag="gr", name=f"gr_{nb}")
            nc.vector.reciprocal(gr[:], ua[:])
            nc.vector.tensor_copy(g_t[nb][:], gr[:])
            if nb == NNB - 1:   # zero g on the 120 pad rows
                nc.vector.tensor_tensor(g_t[nb][:], g_t[nb][:],
                                        onesmask_t[:, nb:nb + 1], Alu.mult)

        # ---------------- phase F chunk ---------------------------------

        ev_grp = [None]

        def emit_f_front(fidx, nb, j, ev_eng, tag):
            jsl = slice(j * MT, (j + 1) * MT)
            nsl = slice(nb * NBLK, (nb + 1) * NBLK)
            if tag == "uR":
                ft2 = pz.tile([NBLK, 2, BANKW], dt.float32, tag="uR",
                              bufs=2, name=f"fr_{nb}_{j}")
                ft = ft2[:, 0, :]
            else:
                ft = pz.tile([NBLK, BANKW], dt.float32, tag=tag,
                             bufs=2, name=f"f_{nb}_{j}")[:]
            nc.tensor.matmul(ft[0:NBLK, 0:MT], f8av[:, :, nsl],
                             f8bv[:, :, jsl], start=True, stop=True,
                             perf_mode=DR)
            # ev tiles stage in groups of 8 and ship to HBM as ONE DMA per
            # group (HWDGE issue overhead is per-instruction); U_v column
            # sums finish on the host.
            slot = fidx % 8
            if slot == 0:
                ev_grp[0] = fwkpool.tile([NBLK, 8, MT], dt.bfloat16, tag="ev",
                                         bufs=3, name=f"evg_{fidx // 8}")
            evb = ev_grp[0][:, slot, :]
            ev_i = evb.bitcast(dt.int16)
            if ev_eng == "act":
                nc.scalar.activation(evb, ft[0:NBLK, 0:MT], Act.Exp)
            elif ev_eng == "pool":
                nc.gpsimd.tensor_scalar(ev_i, ft[0:NBLK, 0:MT], SA, SB,
                                        Alu.mult, Alu.add)
            else:
                nc.vector.tensor_scalar(ev_i, ft[0:NBLK, 0:MT], SA, SB,
                                        Alu.mult, Alu.add)
            if slot == 7:
                nc.sync.dma_start(ev_out[fidx // 8],
                                  ev_grp[0][:].rearrange("p a m -> p (a m)"))
            # t = eh * ev (SBUF-only, so Pool can carry most of these)
            t_t = fwkpool.tile([NBLK, MT], dt.bfloat16, tag="t",
                               bufs=_EVBUFS, name=f"t_{nb}_{j}")
            teng = nc.gpsimd if next(t_eng_cycle) == "pool" else nc.vector
            teng.tensor_tensor(t_t[:], eh_big[:, nb, jsl], evb,
                               Alu.mult)
            return (nb, j, t_t)

        def emit_f_mvs(front):
            nb, j, t_t = front
            jj = j % 4
            obank = oaccA if j < 4 else oaccB
            orow = obank[32 * jj:32 * jj + 1, 0:MT]
            nc.tensor.matmul(orow, g_t[nb][:], t_t[:],
                             start=(nb == 0), stop=(nb == NNB - 1),
                             skip_group_check=True,
                             tile_position=(0, 32 * jj))
            if nb == NNB - 1:
                emit_group_drain(j // 4, jj)

        osb = fwkpool.tile([128, 2, MT], dt.float32, tag="osb", bufs=1,
                           name="osb")
        grp_done = [0, 0]

        def emit_group_drain(grp, jj):
            # row jj of bank grp finished; after all 4, copy + DMA the rows.
            grp_done[grp] += 1
            if grp_done[grp] < 4:
                return
            bank = oaccA if grp == 0 else oaccB
            nc.vector.tensor_copy(osb[:, grp, :], bank[:, 0:MT])
            ov = osb[:, grp, :].rearrange("(q t) m -> q t m", t=32)[:, 0, :]
            nc.sync.dma_start(o_out[grp], ov)

        # ---------------- schedule --------------------------------------
        f_next = 0
        pending = []       # emitted fronts awaiting their matvecs
        g_done = -1
        r_emitted = 0
        t_eng_cycle = itertools.cycle(_T_ENG)
        tail_eng = itertools.cycle(_TAIL_ENG)
        tag_post = itertools.cycle(["uF"])

        def f_ready():
            if f_next >= len(FQUEUE):
                return False
            nb, j = FQUEUE[f_next]
            return g_done >= nb

        def f_slot(during_r, tag_r="uF"):
            nonlocal f_next
            if (len(pending) >= _LAG + (1 if f_ready() else 0)) or \
                    (pending and not f_ready()):
                emit_f_mvs(pending.pop(0))
            if f_ready():
                nb, j = FQUEUE[f_next]
                if f_next >= _EV_ACT_TAIL:
                    eng = next(tail_eng)
                elif _MID_ACT_STRIDE and f_next % _MID_ACT_STRIDE == \
                        _MID_ACT_STRIDE - 1:
                    eng = "act"
                else:
                    eng = "dve"
                tag = tag_r if during_r else next(tag_post)
                pending.append(emit_f_front(f_next, nb, j, eng, tag))
                f_next += 1

        for nb in range(NNB):
            uhp = []
            for ci in range(4):
                emit_r_chunk(nb, ci, uhp)
                for _ in range(_FSLOT_PAT[r_emitted % len(_FSLOT_PAT)]):
                    f_slot(during_r=True)
                if _URCHAIN and r_emitted % 2 == 1:
                    f_slot(during_r=True, tag_r="uR")
                r_emitted += 1
            emit_r_gfin(nb, uhp)
            g_done = nb
        while f_next < len(FQUEUE) or pending:
            f_slot(during_r=False)

    nc.compile()
    return nc


def _get_nc(s):
    if s not in _CACHED:
        _CACHED[s] = _build(s)
    return _CACHED[s]


# ----------------------------------------------------------------------------
# Entry point
# ----------------------------------------------------------------------------

def kernel(**inputs):
    from concourse.bass_utils import run_bass_kernel_spmd

    in_maps = _host_prep(inputs)

    # One program for all 8 cores: the sample/row-half each core handles is
    # fully encoded in its host-built feature tiles.
    nc = _get_nc(0)
    last_err = None
    for attempt in range(3):
        try:
            r = run_bass_kernel_spmd(nc, in_maps, core_ids=list(range(NCORES)))
            break
        except Exception as e:  # transient NRT_EXEC_UNIT_UNRECOVERABLE wedges
            last_err = e
            import time
            time.sleep(10 * (attempt + 1))
    else:
        raise last_err
    results = r.results

    # host combine (exact)
    def _gather_o(arr):
        out = np.zeros(N, np.float64)
        for s, js in enumerate(SWEEPJS):
            for p, j in enumerate(js):
                out[j * MT:(j + 1) * MT] = arr[s, p].astype(np.float64)
        return out

    def _uv_from_ev(ev):
        """ev [NFC//8, 128, 8, MT] bf16 -> U_v partial [N] (core's rows)."""
        uv = np.zeros(N, np.float64)
        e = ev.astype(np.float64)
        for fidx, (nb, j) in enumerate(FQUEUE):
            jsl = slice(j * MT, (j + 1) * MT)
            tile = e[fidx // 8, :, (fidx % 8) * MT:(fidx % 8 + 1) * MT]
            if nb == NNB - 1:   # only partitions 0..7 are real rows
                uv[jsl] += tile[0:8].sum(axis=0)
            else:
                uv[jsl] += tile.sum(axis=0)
        return uv

    logs = np.zeros((B, N), np.float64)
    for b in range(B):
        r0, r1 = results[2 * b], results[2 * b + 1]
        O = _gather_o(r0["o_out"]) + _gather_o(r1["o_out"])
        uv = _uv_from_ev(r0["ev_out"]) + _uv_from_ev(r1["ev_out"])
        res_sum = O / uv
        logs[b] = np.log(res_sum + 1e-4)
    return np.float32(logs.mean())


# revision 27
# speedup vs baseline: 1.1698x; 1.0019x over previous
"""Trainium2 Bass kernel for nn_FMAPModelWarping (retrieval_knn).

The host does the cheap per-pixel prep (affine grids, bilinear taps, the
3x3x3->64 and 1x1 convs, 4-tap backward warp — ~1 GFLOP total, <4% of the
model) and ships fp8 feature maps. The 8 NeuronCores do the FLOP-heavy
part (~26.5 GFLOP): two 3600x3600x128 correlations per sample and the
bidirectional-softmax reduction, tiled flash-attention-style.

Sharding: core k = 2*b + s handles sample b (of 4) and row-half s of the
3600x3600 correlation matrices; partial column stats combine on the host.

Math restructure (exact):
  g[n] = 1/U_h[n],  res_sum[m] = O[m] / U_v[m],
  O[m] = sum_n g[n] * eh[n,m] * ev[n,m]
with U_h = rowsum(eh), U_v = colsum(ev), eh = exp(Mh), ev = exp(Mv).

Device structure: features live in a dual-plane [64, 2, n] fp8 layout so
the correlation matmuls run in DoubleRow perf mode.  Phase R (ACT-bound)
computes exp(Mh) row-blocks into SBUF on its own 4-bank double-buffered
PSUM rotation — no other engine ever blocks it.  Phase F computes exp(Mv)
per (row-pair, m-tile) chunk — Schraudolph bit-trick exponentials on Pool
(+DVE/ACT after R drains) — forms t = eh*ev on DVE, reduces O over rows
via PSUM-accumulated PE matvecs (all 8 m-tiles live at once, 2 banks),
and ships the raw bf16 exp(Mv) tiles to HBM over the otherwise-idle DMA
engines: the column sums U_v are finished on the host, which removes the
whole U_v reduction from the device's critical engines.  The Schraudolph
noise cancels between O and U_v since both consume identical ev values.
"""

import numpy as np

B, C_IN, H, W = 4, 3, 60, 60
HID, FEAT = 64, 128
N = H * W               # 3600
NCORES = 8
HALF = N // 2           # 1800 rows per core
NBLK = 128              # correlation row-block (partition dim)
NNB = 15                # row blocks per core (15*128 = 1920, rows padded)
NPAD = NNB * NBLK       # 1920
MT = 450                # m-tile width
N_MT = N // MT          # 8 m tiles
BANKW = 512             # fp32 elems per PSUM bank
NFC = 120               # F chunks: 15 row-blocks x 8 m-tiles

# Schraudolph constants (bf16 target): i16 = rne(x*SA + SB); bits as bf16.
SA = 128.0 / float(np.log(2.0))
SB = 127.0 * 128.0 - 5.5 - 1.86   # -1.86 centers the measured +1% bias

# m-tile groups: bank A holds O rows for j=0..3, bank B for j=4..7 (4 rows
# per bank at partition offsets 0/32/64/96); the same grouping indexes the
# [2, 4, MT] output.
SWEEPJS = [(0, 1, 2, 3), (4, 5, 6, 7)]

# F-chunk queue: row-block-major, m-tile-minor (all 8 m-tiles' O rows
# accumulate concurrently in two PSUM banks).
FQUEUE = [(nb, j) for nb in range(NNB) for j in range(8)]

# scheduling knobs (tuned via cost-model sweeps)
_FSLOT_PAT = (2,)       # F slots per R chunk, cycled
_LAG = 20       # fronts in flight before their matvecs are emitted
_EVBUFS = 24    # ev/t tile rotation depth
# Pool cannot read PSUM (BIR verifier), so exp(Mv) runs on DVE with the
# tail of the chunk queue on ACT (free once phase R drains); Pool instead
# carries most of the SBUF-only t=eh*ev multiplies.
_EV_ACT_TAIL = 80      # chunks with fidx >= this rotate through _TAIL_ENG
_TAIL_ENG = ["act", "dve", "dve"]
_MID_ACT_STRIDE = 8    # if >0, every Nth pre-tail chunk exps on ACT
_URCHAIN = True        # third F chain through uR's banks
_T_ENG = ["pool", "pool", "dve"]         # t-mult engine cycle


# ----------------------------------------------------------------------------
# Host-side prep: exact reference semantics for grids / bilinear taps / rolls
# ----------------------------------------------------------------------------

def _affine_coords(theta2x3):
    """Pixel-space sample coords (x, y) for torch affine_grid+grid_sample
    (align_corners=False), shape [H, W] each."""
    xs = (2.0 * np.arange(W, dtype=np.float64) + 1.0) / W - 1.0
    ys = (2.0 * np.arange(H, dtype=np.float64) + 1.0) / H - 1.0
    gx, gy = np.meshgrid(xs, ys)           # gx[i,j]=xs[j], gy[i,j]=ys[i]
    t = theta2x3.astype(np.float64)
    cx = t[0, 0] * gx + t[0, 1] * gy + t[0, 2]
    cy = t[1, 0] * gx + t[1, 1] * gy + t[1, 2]
    px = (cx + 1.0) * W * 0.5 - 0.5
    py = (cy + 1.0) * H * 0.5 - 0.5
    return px, py


def _bilinear_sample_host(img, px, py):
    """img [C,H,W] float32, sample at (px,py) [H,W]; zeros padding.
    Mirrors reference grid_sample exactly."""
    x0 = np.floor(px); y0 = np.floor(py)
    wx1 = (px - x0); wx0 = 1.0 - wx1
    wy1 = (py - y0); wy0 = 1.0 - wy1
    out = np.zeros((img.shape[0],) + px.shape, np.float64)
    flat = img.reshape(img.shape[0], -1).astype(np.float64)
    for ix, iy, wt in ((x0, y0, wx0 * wy0), (x0 + 1, y0, wx1 * wy0),
                       (x0, y0 + 1, wx0 * wy1), (x0 + 1, y0 + 1, wx1 * wy1)):
        valid = (ix >= 0) & (ix < W) & (iy >= 0) & (iy < H)
        ii = np.clip(ix, 0, W - 1).astype(np.int64)
        jj = np.clip(iy, 0, H - 1).astype(np.int64)
        v = flat[:, (jj * W + ii).ravel()].reshape(out.shape)
        out += v * (wt * valid)[None]
    return out.astype(np.float32)


def _back_taps(theta2x3, u, v):
    """Tap indices/weights for grid_sample(y, grid(Bm)) composed with the
    inverse roll. Returns idx [4,3600] int (in-range), wt [4,3600] f32."""
    px, py = _affine_coords(theta2x3)
    ii = np.arange(H)[:, None]; jj = np.arange(W)[None, :]
    qi = (ii - u) % H; qj = (jj - v) % W
    xs = px[qi, qj].ravel(); ys = py[qi, qj].ravel()
    x0 = np.floor(xs); y0 = np.floor(ys)
    fx = xs - x0; fy = ys - y0
    idxs, wts = [], []
    for ix, iy, wt in ((x0, y0, (1 - fx) * (1 - fy)), (x0 + 1, y0, fx * (1 - fy)),
                       (x0, y0 + 1, (1 - fx) * fy), (x0 + 1, y0 + 1, fx * fy)):
        valid = (ix >= 0) & (ix < W) & (iy >= 0) & (iy < H)
        cii = np.clip(ix, 0, W - 1).astype(np.int64)
        cjj = np.clip(iy, 0, H - 1).astype(np.int64)
        idxs.append(cjj * W + cii)
        wts.append((wt * valid).astype(np.float32))
    return np.stack(idxs), np.stack(wts)


def _host_prep(inputs):
    """Build the 8 per-core device input dicts (fp8 dual-plane features)."""
    import ml_dtypes
    x_a = np.asarray(inputs["input_a"], np.float32)
    x_b = np.asarray(inputs["input_b"], np.float32)
    w1 = np.asarray(inputs["w1"], np.float32)
    b1 = np.asarray(inputs["b1"], np.float32)
    w2 = np.asarray(inputs["w2"], np.float32)
    b2 = np.asarray(inputs["b2"], np.float32)
    noise = np.asarray(inputs["noise"], np.float32)
    u_roll = np.asarray(inputs["u_roll"])
    v_roll = np.asarray(inputs["v_roll"])
    swap = np.asarray(inputs["swap"])

    w1mat = w1.reshape(HID, C_IN * 9)                  # [64, 27]
    w2mat = w2.reshape(FEAT, HID)                      # [128, 64]

    eye = np.eye(3, dtype=np.float64)
    mask = np.array([[1., 1., 1.], [1., 1., 1.], [0., 0., 0.]])

    # F[wrp][b]: warped feature map [FEAT, N] float32 (exact reference math;
    # the 1x1 conv2 commutes with the backward spatial gather)
    F = np.zeros((4, B, FEAT, N), np.float32)
    for wrp in range(4):
        sw = int(swap[wrp]) == 1
        for b in range(B):
            fwd = eye + 0.05 * noise[wrp, b].astype(np.float64) * mask
            bwd = np.linalg.inv(fwd)
            A_ = bwd if sw else fwd
            Bm = fwd if sw else bwd
            u = int(u_roll[wrp, b]); v = int(v_roll[wrp, b])
            img = x_a[b] if wrp in (0, 2) else x_b[b]
            x_r = np.roll(np.roll(img, -u, axis=1), -v, axis=2)
            px, py = _affine_coords(np.asarray(A_)[:2])
            xw = _bilinear_sample_host(x_r, px, py)       # [3,60,60]
            # im2col, zero-pad SAME, k = c*9 + ky*3 + kx
            pad = np.zeros((C_IN, H + 2, W + 2), np.float32)
            pad[:, 1:-1, 1:-1] = xw
            X1 = np.zeros((C_IN * 9, N), np.float32)
            k = 0
            for c in range(C_IN):
                for ky in range(3):
                    for kx in range(3):
                        X1[k] = pad[c, ky:ky + H, kx:kx + W].ravel()
                        k += 1
            y1 = np.maximum(w1mat @ X1 + b1[:, None], 0.0)   # [64, N]
            y2 = w2mat @ y1 + b2[:, None]                    # [128, N]
            idx, wt = _back_taps(np.asarray(Bm)[:2], u, v)
            Fw = np.zeros((FEAT, N), np.float32)
            for tap in range(4):
                Fw += y2[:, idx[tap]] * wt[tap][None, :]
            F[wrp, b] = Fw

    F8 = F.astype(ml_dtypes.float8_e4m3fn)

    def dual_plane(feat, cols):
        """[FEAT, n] -> [64, 2, cols] (zero-padded)."""
        out = np.zeros((64, 2, cols), ml_dtypes.float8_e4m3fn)
        n = feat.shape[1]
        out[:, 0, :n] = feat[0:64]
        out[:, 1, :n] = feat[64:128]
        return out

    # g-mask stationary: ones, except block 14 masks the 120 pad rows
    onesmask = np.ones((128, NNB), np.float32)
    onesmask[8:, NNB - 1] = 0.0

    in_maps = []
    for core in range(NCORES):
        b = core // 2
        s = core % 2
        n0 = s * HALF
        in_maps.append({
            "f8ah_in": dual_plane(F8[0, b][:, n0:n0 + HALF], NPAD),
            "f8av_in": dual_plane(F8[2, b][:, n0:n0 + HALF], NPAD),
            "f8bh_in": dual_plane(F8[1, b], N),
            "f8bv_in": dual_plane(F8[3, b], N),
            "onesmask_in": onesmask.astype(ml_dtypes.bfloat16),
        })
    return in_maps


# ----------------------------------------------------------------------------
# Device kernel builder
# ----------------------------------------------------------------------------

_CACHED = {}


def _build(core_half):
    """Build the Bacc module (one NEFF shared by all 8 cores; each core's
    sample/row-half is fully encoded in its host-built feature tiles)."""
    import concourse.bacc as bacc_mod
    import concourse.mybir as mybir
    from concourse.tile import TileContext
    from contextlib import ExitStack
    import itertools

    dt = mybir.dt
    Alu = mybir.AluOpType
    Act = mybir.ActivationFunctionType
    DR = mybir.MatmulPerfMode.DoubleRow

    nc = bacc_mod.Bacc("TRN2", target_bir_lowering=False)

    f8ah_in = nc.dram_tensor("f8ah_in", [64, 2, NPAD], dt.float8e4, kind="ExternalInput")
    f8av_in = nc.dram_tensor("f8av_in", [64, 2, NPAD], dt.float8e4, kind="ExternalInput")
    f8bh_in = nc.dram_tensor("f8bh_in", [64, 2, N], dt.float8e4, kind="ExternalInput")
    f8bv_in = nc.dram_tensor("f8bv_in", [64, 2, N], dt.float8e4, kind="ExternalInput")
    onesmask_in = nc.dram_tensor("onesmask_in", [FEAT, NNB], dt.bfloat16, kind="ExternalInput")

    o_out = nc.dram_tensor("o_out", [2, 4, MT], dt.float32, kind="ExternalOutput")
    ev_out = nc.dram_tensor("ev_out", [NFC // 8, NBLK, 8 * MT], dt.bfloat16, kind="ExternalOutput")

    with ExitStack() as ctx:
        tc = ctx.enter_context(TileContext(nc))

        const = ctx.enter_context(tc.tile_pool(name="const", bufs=1))
        onesmask_t = const.tile([FEAT, NNB], dt.bfloat16)
        nc.sync.dma_start(onesmask_t[:], onesmask_in[:])

        fpool = ctx.enter_context(tc.tile_pool(name="feat", bufs=1))
        f8ah = fpool.tile([64, 2, NPAD], dt.float8e4, name="f8ah")
        f8av = fpool.tile([64, 2, NPAD], dt.float8e4, name="f8av")
        f8bh = fpool.tile([64, 2, N], dt.float8e4, name="f8bh")
        f8bv = fpool.tile([64, 2, N], dt.float8e4, name="f8bv")
        nc.sync.dma_start(f8ah[:], f8ah_in[:])
        nc.sync.dma_start(f8bh[:], f8bh_in[:])
        nc.sync.dma_start(f8av[:], f8av_in[:])
        nc.sync.dma_start(f8bv[:], f8bv_in[:])

        stat = ctx.enter_context(tc.tile_pool(name="stat", bufs=1))
        eh_big = stat.tile([NBLK, NNB, N], dt.bfloat16, name="ehbig")
        g_t = [stat.tile([NBLK, 1], dt.bfloat16, name=f"g{_nb}")
               for _nb in range(NNB)]

        rs = ctx.enter_context(tc.tile_pool(name="rsmall", bufs=10))

        # PSUM (8 banks): uR = R-phase rotation, 2-bank chunks x 2 bufs
        # (4 banks) — R's ACT stream never waits on any other engine; uF =
        # F-phase rotation, 2-bank chunks, single buf during R (serial,
        # Pool-paced) — after R retires, F chunks also cycle through uR's
        # freed banks for a 3-deep rotation; oaccA/oaccB hold the 8 live
        # O-accumulator rows (4 rows per bank at partition offsets
        # 0/32/64/96) so the whole F phase runs c-major with no
        # inter-m-tile serialization.
        pz = ctx.enter_context(tc.tile_pool(name="pz", bufs=1, space="PSUM"))
        fwkpool = ctx.enter_context(tc.tile_pool(name="fwkpool", bufs=1))

        oaccA = pz.tile([NBLK, BANKW], dt.float32, tag="accA", name="oaccA")
        oaccB = pz.tile([NBLK, BANKW], dt.float32, tag="accB", name="oaccB")

        # R chunking: 4 chunks per nb, 2 m-tiles (900 cols) each
        def emit_r_chunk(nb, ci, uhp):
            nsl = slice(nb * NBLK, (nb + 1) * NBLK)
            m0 = ci * 2 * MT
            rt = pz.tile([NBLK, 2, BANKW], dt.float32, tag="uR", bufs=2,
                         name=f"r_{nb}_{ci}")
            for k in range(2):
                nc.tensor.matmul(rt[:, k, 0:MT], f8ah[:, :, nsl],
                                 f8bh[:, :, m0 + k * MT: m0 + (k + 1) * MT],
                                 start=True, stop=True, perf_mode=DR)
            ehv = eh_big[:, nb, m0: m0 + 2 * MT] \
                .rearrange("p (c w) -> p c w", w=MT)
            uh = rs.tile([NBLK, 1], dt.float32, tag=f"uh{ci}",
                         name=f"uh_{nb}_{ci}")
            nc.scalar.activation(ehv, rt[:, 0:2, 0:MT], Act.Exp,
                                 accum_out=uh[:])
            uhp.append(uh)

        def emit_r_gfin(nb, uhp):
            ua = rs.tile([NBLK, 1], dt.float32, tag="ua", name=f"ua_{nb}")
            nc.vector.tensor_tensor(ua[:], uhp[0][:], uhp[1][:], Alu.add)
            nc.vector.tensor_tensor(ua[:], ua[:], uhp[2][:], Alu.add)
            nc.vector.tensor_tensor(ua[:], ua[:], uhp[3][:], Alu.add)

            gr = rs.tile([NBLK, 1], dt.float32, tag="gr", name=f"gr_{nb}")
            nc.vector.reciprocal(gr[:], ua[:])
            nc.vector.tensor_copy(g_t[nb][:], gr[:])
            if nb == NNB - 1:   # zero g on the 120 pad rows
                nc.vector.tensor_tensor(g_t[nb][:], g_t[nb][:],
                                        onesmask_t[:, nb:nb + 1], Alu.mult)

        # ---------------- phase F chunk ---------------------------------

        ev_grp = [None]

        def emit_f_front(fidx, nb, j, ev_eng, tag):
            jsl = slice(j * MT, (j + 1) * MT)
            nsl = slice(nb * NBLK, (nb + 1) * NBLK)
            if tag == "uR":
                ft2 = pz.tile([NBLK, 2, BANKW], dt.float32, tag="uR",
                              bufs=2, name=f"fr_{nb}_{j}")
                ft = ft2[:, 0, :]
            else:
                ft = pz.tile([NBLK, BANKW], dt.float32, tag=tag,
                             bufs=2, name=f"f_{nb}_{j}")[:]
            nc.tensor.matmul(ft[0:NBLK, 0:MT], f8av[:, :, nsl],
                             f8bv[:, :, jsl], start=True, stop=True,
                             perf_mode=DR)
            # ev tiles stage in groups of 8 and ship to HBM as ONE DMA per
            # group (HWDGE issue overhead is per-instruction); U_v column
            # sums finish on the host.
            slot = fidx % 8
            if slot == 0:
                ev_grp[0] = fwkpool.tile([NBLK, 8, MT], dt.bfloat16, tag="ev",
                                         bufs=3, name=f"evg_{fidx // 8}")
            evb = ev_grp[0][:, slot, :]
            ev_i = evb.bitcast(dt.int16)
            if ev_eng == "act":
                nc.scalar.activation(evb, ft[0:NBLK, 0:MT], Act.Exp)
            elif ev_eng == "pool":
                nc.gpsimd.tensor_scalar(ev_i, ft[0:NBLK, 0:MT], SA, SB,
                                        Alu.mult, Alu.add)
            else:
                nc.vector.tensor_scalar(ev_i, ft[0:NBLK, 0:MT], SA, SB,
                                        Alu.mult, Alu.add)
            if slot == 7:
                nc.sync.dma_start(ev_out[fidx // 8],
                                  ev_grp[0][:].rearrange("p a m -> p (a m)"))
            # t = eh * ev (SBUF-only, so Pool can carry most of these)
            t_t = fwkpool.tile([NBLK, MT], dt.bfloat16, tag="t",
                               bufs=_EVBUFS, name=f"t_{nb}_{j}")
            teng = nc.gpsimd if next(t_eng_cycle) == "pool" else nc.vector
            teng.tensor_tensor(t_t[:], eh_big[:, nb, jsl], evb,
                               Alu.mult)
            return (nb, j, t_t)

        def emit_f_mvs(front):
            nb, j, t_t = front
            jj = j % 4
            obank = oaccA if j < 4 else oaccB
            orow = obank[32 * jj:32 * jj + 1, 0:MT]
            nc.tensor.matmul(orow, g_t[nb][:], t_t[:],
                             start=(nb == 0), stop=(nb == NNB - 1),
                             skip_group_check=True,
                             tile_position=(0, 32 * jj))
            if nb == NNB - 1:
                emit_group_drain(j // 4, jj)

        osb = fwkpool.tile([128, 2, MT], dt.float32, tag="osb", bufs=1,
                           name="osb")
        grp_done = [0, 0]

        def emit_group_drain(grp, jj):
            # row jj of bank grp finished; after all 4, copy + DMA the rows.
            grp_done[grp] += 1
            if grp_done[grp] < 4:
                return
            bank = oaccA if grp == 0 else oaccB
            nc.vector.tensor_copy(osb[:, grp, :], bank[:, 0:MT])
            ov = osb[:, grp, :].rearrange("(q t) m -> q t m", t=32)[:, 0, :]
            nc.sync.dma_start(o_out[grp], ov)

        # ---------------- schedule --------------------------------------
        f_next = 0
        pending = []       # emitted fronts awaiting their matvecs
        g_done = -1
        r_emitted = 0
        t_eng_cycle = itertools.cycle(_T_ENG)
        tail_eng = itertools.cycle(_TAIL_ENG)
        tag_post = itertools.cycle(["uF"])

        def f_ready():
            if f_next >= len(FQUEUE):
                return False
            nb, j = FQUEUE[f_next]
            return g_done >= nb

        def f_slot(during_r, tag_r="uF"):
            nonlocal f_next
            if (len(pending) >= _LAG + (1 if f_ready() else 0)) or \
                    (pending and not f_ready()):
                emit_f_mvs(pending.pop(0))
            if f_ready():
                nb, j = FQUEUE[f_next]
                if f_next >= _EV_ACT_TAIL:
                    eng = next(tail_eng)
                elif _MID_ACT_STRIDE and f_next % _MID_ACT_STRIDE == \
                        _MID_ACT_STRIDE - 1:
                    eng = "act"
                else:
                    eng = "dve"
                tag = tag_r if during_r else next(tag_post)
                pending.append(emit_f_front(f_next, nb, j, eng, tag))
                f_next += 1

        for nb in range(NNB):
            uhp = []
            for ci in range(4):
                emit_r_chunk(nb, ci, uhp)
                for _ in range(_FSLOT_PAT[r_emitted % len(_FSLOT_PAT)]):
                    f_slot(during_r=True)
                if _URCHAIN and r_emitted % 2 == 1:
                    f_slot(during_r=True, tag_r="uR")
                r_emitted += 1
            emit_r_gfin(nb, uhp)
            g_done = nb
        while f_next < len(FQUEUE) or pending:
            f_slot(during_r=False)

    nc.compile()
    return nc


def _get_nc(s):
    if s not in _CACHED:
        _CACHED[s] = _build(s)
    return _CACHED[s]


# ----------------------------------------------------------------------------
# Entry point
# ----------------------------------------------------------------------------

def kernel(**inputs):
    from concourse.bass_utils import run_bass_kernel_spmd

    in_maps = _host_prep(inputs)

    # One program for all 8 cores: the sample/row-half each core handles is
    # fully encoded in its host-built feature tiles.
    nc = _get_nc(0)
    last_err = None
    for attempt in range(3):
        try:
            r = run_bass_kernel_spmd(nc, in_maps, core_ids=list(range(NCORES)))
            break
        except Exception as e:  # transient NRT_EXEC_UNIT_UNRECOVERABLE wedges
            last_err = e
            import time
            time.sleep(10 * (attempt + 1))
    else:
        raise last_err
    results = r.results

    # host combine (exact)
    def _gather_o(arr):
        out = np.zeros(N, np.float64)
        for s, js in enumerate(SWEEPJS):
            for p, j in enumerate(js):
                out[j * MT:(j + 1) * MT] = arr[s, p].astype(np.float64)
        return out

    def _uv_from_ev(ev):
        """ev [NFC//8, 128, 8, MT] bf16 -> U_v partial [N] (core's rows)."""
        uv = np.zeros(N, np.float64)
        e = ev.astype(np.float64)
        for fidx, (nb, j) in enumerate(FQUEUE):
            jsl = slice(j * MT, (j + 1) * MT)
            tile = e[fidx // 8, :, (fidx % 8) * MT:(fidx % 8 + 1) * MT]
            if nb == NNB - 1:   # only partitions 0..7 are real rows
                uv[jsl] += tile[0:8].sum(axis=0)
            else:
                uv[jsl] += tile.sum(axis=0)
        return uv

    logs = np.zeros((B, N), np.float64)
    for b in range(B):
        r0, r1 = results[2 * b], results[2 * b + 1]
        O = _gather_o(r0["o_out"]) + _gather_o(r1["o_out"])
        uv = _uv_from_ev(r0["ev_out"]) + _uv_from_ev(r1["ev_out"])
        res_sum = O / uv
        logs[b] = np.log(res_sum + 1e-4)
    return np.float32(logs.mean())
